# revision 11
# baseline (speedup 1.0000x reference)
"""Trainium2 Bass kernel for nn_AttentionNetwork (temporal attention pooling).

Reference computation (B=4, F=256, T=8192, H=1024, C=128):
    z         = einsum("bft,fh->bth", seq, Wb) + bb          [B,T,H]
    logits    = z @ Wa + ba                                   [B,T,C]
    attention = softmax(logits, axis=2) / T                   [B,T,C]
    rep       = einsum("bth,btc->bhc", z, attention)          [B,H,C]
    action    = einsum("bhc,hc->bc", rep, A) + action_bias    [B,C]
    thres     = (rep.transpose(0,2,1) @ Wt)[...,0] + bt       [B,C]

Sharding: 8 cores = 4 batch x 2 T-halves (T_loc = 4096 per core).
Each core computes its attention slice and the partial (T-half)
rep0^T = att^T @ z0 where z0 = seq^T @ Wb (no bias). The host sums the
two partials, applies the rank-1 bias correction
rep += outer(sum_t att, bb), and runs the tiny linear epilogue
(action/thres) -- everything downstream of rep is linear in it.

Algebraic refactors keeping all device matmuls in the [t,*] orientation:
  logits = seq^T @ (Wb@Wa) + (bb@Wa + ba)   (fused on host)
  rep    = att^T @ z0 + outer(sum_t att, bb) (corrected on host)

Matmuls run as float32r (fp32 stored, fp22 multiply, fp32 accumulate) --
4x the fp32 matmul rate on the PE array.
"""

import numpy as np

import concourse.bacc as bacc
import concourse.mybir as mybir
import concourse.tile as tile
from concourse.bass_utils import run_bass_kernel_spmd

B, F, T, H, C = 4, 256, 8192, 1024, 128
NCORES = 8
TSPLIT = NCORES // B          # 2 T-shards per batch element
TLOC = T // TSPLIT            # 4096 timesteps per core
PT = 128                      # t-tile (partition dim)
NT = TLOC // PT               # 32 t-tiles
FK = F // 128                 # 2 contraction tiles over F
HB = 512                      # h-chunk per matmul (one PSUM bank, fp32)
NSEQ_CHUNKS = 8               # DMA pipelining chunks for the seq load

F32 = mybir.dt.float32
F32R = mybir.dt.float32r      # fp22 multiply / fp32 accumulate on PE
C2 = 2 * C                    # logits N padded to 256 (fp32r needs N>=256
                              # for the 1 cyc/row fast path; Wf cols duplicated)


def build_nc():
    nc = bacc.Bacc(trn_type="TRN2")

    # Per-core inputs (host pre-shards / pre-broadcasts).
    seq_s = nc.dram_tensor("seq_s", [F, TLOC], F32R, kind="ExternalInput")
    wb = nc.dram_tensor("wb", [F, H], F32R, kind="ExternalInput")
    wf = nc.dram_tensor("wf", [F, C2], F32R, kind="ExternalInput")
    expbf_bc = nc.dram_tensor("expbf_bc", [128, C], F32, kind="ExternalInput")

    att_out = nc.dram_tensor("att_out", [TLOC, C], F32R, kind="ExternalOutput")
    rep_out = nc.dram_tensor("rep_out", [C, H], F32, kind="ExternalOutput")

    with tile.TileContext(nc) as tc:
        with (
            tc.tile_pool(name="consts", bufs=1) as consts,
            tc.tile_pool(name="zpool", bufs=3) as zpool,
            tc.tile_pool(name="small", bufs=4) as small,
            tc.tile_pool(name="psz", bufs=2, space="PSUM") as psz,
            tc.tile_pool(name="pslg", bufs=2, space="PSUM") as pslg,
            tc.tile_pool(name="psrep", bufs=1, space="PSUM") as psrep,
        ):
            # ---- constant loads -------------------------------------------
            wb_sb = consts.tile([128, FK, H], F32R)
            nc.sync.dma_start(out=wb_sb, in_=wb.rearrange("(k p) h -> p k h", p=128))
            wf_sb = consts.tile([128, FK, C2], F32R)
            nc.sync.dma_start(out=wf_sb, in_=wf.rearrange("(k p) c -> p k c", p=128))
            expbf_sb = consts.tile([128, C], F32)
            nc.sync.dma_start(out=expbf_sb, in_=expbf_bc[:, :])

            # seq resident in SBUF, loaded in chunks so compute can start early
            seq_sb = consts.tile([128, FK, TLOC], F32R)
            seq_src = seq_s.rearrange("(k p) t -> p k t", p=128)
            tchunk = TLOC // NSEQ_CHUNKS
            for ci in range(NSEQ_CHUNKS):
                sl = slice(ci * tchunk, (ci + 1) * tchunk)
                nc.sync.dma_start(out=seq_sb[:, :, sl], in_=seq_src[:, :, sl])

            # rep accumulator lives in PSUM across the whole t-loop
            ps_rep = psrep.tile([C, H], F32)

            # ---- main loop over 32 t-tiles --------------------------------
            for i in range(NT):
                ts = slice(i * PT, (i + 1) * PT)

                ps_z = psz.tile([PT, H], F32)
                ps_lg = pslg.tile([PT, C2], F32)
                for k in range(FK):
                    st, sp = (k == 0), (k == FK - 1)
                    for hb in range(H // HB):
                        hs = slice(hb * HB, (hb + 1) * HB)
                        nc.tensor.matmul(
                            ps_z[:, hs], seq_sb[:, k, ts], wb_sb[:, k, hs],
                            start=st, stop=sp,
                        )
                    nc.tensor.matmul(
                        ps_lg, seq_sb[:, k, ts], wf_sb[:, k, :],
                        start=st, stop=sp,
                    )

                # evacuate z: plain copies split between ACT and DVE
                z_sb = zpool.tile([PT, H], F32R)
                nc.scalar.copy(z_sb[:, 0:HB], ps_z[:, 0:HB])
                nc.vector.tensor_copy(z_sb[:, HB:H], ps_z[:, HB:H])

                # softmax over classes (free dim) / T:
                # att = exp(lg0)*exp(bf) / sum_c(exp(lg0)*exp(bf)) / T
                e = small.tile([PT, C], F32)
                nc.scalar.activation(
                    e, ps_lg[:, 0:C], mybir.ActivationFunctionType.Exp
                )
                em = small.tile([PT, C], F32)
                nc.gpsimd.tensor_mul(em, e, expbf_sb)
                ssum = small.tile([PT, 1], F32)
                nc.vector.reduce_sum(ssum, em, axis=mybir.AxisListType.X)
                rcp = small.tile([PT, 1], F32)
                nc.vector.reciprocal(rcp, ssum)
                att = small.tile([PT, C], F32R)
                nc.gpsimd.tensor_scalar(
                    att, em, rcp, 1.0 / T,
                    mybir.AluOpType.mult, mybir.AluOpType.mult,
                )
                nc.sync.dma_start(out=att_out[ts, :], in_=att)

                # rep0^T[c,h] += att[t,c].T @ z0[t,h]
                for hb in range(H // HB):
                    hs = slice(hb * HB, (hb + 1) * HB)
                    nc.tensor.matmul(
                        ps_rep[:, hs], att, z_sb[:, hs],
                        start=(i == 0), stop=(i == NT - 1),
                    )

            # ---- epilogue: partial rep out --------------------------------
            rep_sb = consts.tile([C, H], F32)
            nc.scalar.copy(rep_sb, ps_rep)
            nc.sync.dma_start(out=rep_out[:, :], in_=rep_sb)

    nc.finalize()
    return nc


def _prepare_in_maps(seq, Wb, bb, Wa, ba):
    seq = np.ascontiguousarray(np.asarray(seq, dtype=np.float32))
    Wb = np.ascontiguousarray(np.asarray(Wb, dtype=np.float32))
    bb = np.asarray(bb, dtype=np.float32)
    Wa = np.asarray(Wa, dtype=np.float32)
    ba = np.asarray(ba, dtype=np.float32)

    wf1 = (Wb.astype(np.float64) @ Wa.astype(np.float64)).astype(np.float32)
    wf = np.ascontiguousarray(np.concatenate([wf1, wf1], axis=1))  # [F, 2C]
    bf = (bb.astype(np.float64) @ Wa.astype(np.float64)
          + ba.astype(np.float64))
    expbf = np.exp(bf).astype(np.float32)
    expbf_bc = np.ascontiguousarray(np.broadcast_to(expbf[None, :], (128, C)))

    in_maps = []
    for core in range(NCORES):
        b, sh = core // TSPLIT, core % TSPLIT
        t0 = sh * TLOC
        in_maps.append({
            "seq_s": np.ascontiguousarray(seq[b, :, t0:t0 + TLOC]),
            "wb": Wb, "wf": wf, "expbf_bc": expbf_bc,
        })
    return in_maps


def _assemble(results, bb, action_matrix, action_bias, Wt, bt):
    bb = np.asarray(bb, dtype=np.float64)
    A = np.asarray(action_matrix, dtype=np.float64)
    action_bias = np.asarray(action_bias, dtype=np.float64)
    Wt = np.asarray(Wt, dtype=np.float64)
    bt = np.asarray(bt, dtype=np.float64)

    attention = np.empty((B, T, C), dtype=np.float32)
    rep_t = np.zeros((B, C, H), dtype=np.float64)
    for core in range(NCORES):
        r = results[core]
        b, sh = core // TSPLIT, core % TSPLIT
        t0 = sh * TLOC
        attention[b, t0:t0 + TLOC, :] = r["att_out"]
        rep_t[b] += r["rep_out"]

    # rank-1 bias correction: rep^T[c,h] += (sum_t att[t,c]) * bb[h]
    s_att = attention.astype(np.float64).sum(axis=1)          # [B, C]
    rep_t += s_att[:, :, None] * bb[None, None, :]

    rep_feature = np.ascontiguousarray(
        rep_t.transpose(0, 2, 1)).astype(np.float32)          # [B, H, C]
    rep64 = rep_feature.astype(np.float64)
    action_logit = (np.einsum("bhc,hc->bc", rep64, A)
                    + action_bias).astype(np.float32)
    thres = (np.einsum("bhc,h->bc", rep64, Wt[:, 0]) + bt).astype(np.float32)
    return attention, rep_feature, action_logit, thres


def run(inputs, **spmd_kwargs):
    """Build, run on 8 cores, and assemble. Returns (outputs, BassKernelResults)."""
    nc = build_nc()
    in_maps = _prepare_in_maps(
        inputs["seq"], inputs["Wb"], inputs["bb"], inputs["Wa"], inputs["ba"],
    )
    res = run_bass_kernel_spmd(nc, in_maps, core_ids=list(range(NCORES)),
                               **spmd_kwargs)
    outs = _assemble(res.results, inputs["bb"], inputs["action_matrix"],
                     inputs["action_bias"], inputs["Wt"], inputs["bt"])
    return outs, res


def kernel(**inputs):
    outs, _ = run(inputs)
    return outs


# revision 14
# speedup vs baseline: 1.1960x; 1.1960x over previous
"""Trainium2 Bass kernel for nn_AttentionNetwork (temporal attention pooling).

Reference computation (B=4, F=256, T=8192, H=1024, C=128):
    z         = einsum("bft,fh->bth", seq, Wb) + bb          [B,T,H]
    logits    = z @ Wa + ba                                   [B,T,C]
    attention = softmax(logits, axis=2) / T                   [B,T,C]
    rep       = einsum("bth,btc->bhc", z, attention)          [B,H,C]
    action    = einsum("bhc,hc->bc", rep, A) + action_bias    [B,C]
    thres     = (rep.transpose(0,2,1) @ Wt)[...,0] + bt       [B,C]

Sharding: 8 cores = 4 batch x 2 T-halves (T_loc = 4096 per core).

Key algebraic refactors (all exact up to fp reassociation):
  1. logits = seq^T @ (Wb@Wa) + (bb@Wa + ba)  -- Wf := Wb@Wa fused on host
     (F=256 contraction instead of H=1024, and z is not needed for logits).
  2. rep    = Wb^T @ (seq @ att) + outer(sum_t att, bb)
     -- contract seq with attention FIRST (matrix-chain reordering):
     M^T[c,f] = sum_t att[t,c] seq[f,t] accumulates tile-by-tile in PSUM,
     then one tiny projection through Wb at the end. z is never
     materialized at all; the host adds the rank-1 bb correction using
     sum_t att (computed from the attention output it already has).
  3. The logits bias rides a K=1 ones-row matmul into PSUM, so the
     softmax reads logits+bias straight from PSUM.

Per-core device work: logits (seq^T@Wf, N padded to 256 for the fp32r
fast path), softmax/T (ACT exp + DVE sum/recip + GPSIMD scale), M^T
accumulation, and the final Wb projection -- ~0.3 G MAC vs 1.74 G for
the naive z-based dataflow.

Matmuls run as float32r (fp32 stored, fp22 multiply, fp32 accumulate) --
4x the fp32 matmul rate on the PE array at moving-dim >= 256.

The host sends seq in BOTH orientations ([F,T_loc] for logits
stationaries, [T_loc,F] for the M^T matmul) -- a transposed copy is
cheaper as DMA than as on-device PE transposes.
"""

import numpy as np

import concourse.bacc as bacc
import concourse.mybir as mybir
import concourse.tile as tile
from concourse.bass_utils import run_bass_kernel_spmd

B, F, T, H, C = 4, 256, 8192, 1024, 128
NCORES = 8
TSPLIT = NCORES // B          # 2 T-shards per batch element
TLOC = T // TSPLIT            # 4096 timesteps per core
PT = 128                      # t-tile (partition dim)
NT = TLOC // PT               # 32 t-tiles
FK = F // 128                 # 2 contraction tiles over F
HB = 512                      # h-chunk per matmul (one PSUM bank, fp32)
NSEQ_CHUNKS = 8               # DMA pipelining chunks for the seq load

F32 = mybir.dt.float32
F32R = mybir.dt.float32r      # fp22 multiply / fp32 accumulate on PE
C2 = 2 * C                    # logits N padded to 256 (fp32r needs N>=256
                              # for the 1 cyc/row fast path; Wf cols duplicated)


def build_nc():
    nc = bacc.Bacc(trn_type="TRN2")

    # Per-core inputs (host pre-shards / pre-transposes / pre-duplicates).
    seq_s = nc.dram_tensor("seq_s", [F, TLOC], F32R, kind="ExternalInput")
    seq_t = nc.dram_tensor("seq_t", [TLOC, F], F32R, kind="ExternalInput")
    wb = nc.dram_tensor("wb", [F, H], F32R, kind="ExternalInput")
    wf = nc.dram_tensor("wf", [F, C2], F32R, kind="ExternalInput")
    bf_dup = nc.dram_tensor("bf_dup", [1, C2], F32R, kind="ExternalInput")
    ident = nc.dram_tensor("ident", [128, 128], F32R, kind="ExternalInput")
    ones_row = nc.dram_tensor("ones_row", [1, PT], F32R, kind="ExternalInput")

    att_out = nc.dram_tensor("att_out", [TLOC, C], F32R, kind="ExternalOutput")
    rep_out = nc.dram_tensor("rep_out", [C, H], F32, kind="ExternalOutput")

    with tile.TileContext(nc) as tc:
        with (
            tc.tile_pool(name="consts", bufs=1) as consts,
            tc.tile_pool(name="small", bufs=6) as small,
            tc.tile_pool(name="pslg", bufs=3, space="PSUM") as pslg,
            tc.tile_pool(name="psm", bufs=1, space="PSUM") as psm,
            tc.tile_pool(name="pst", bufs=2, space="PSUM") as pst,
            tc.tile_pool(name="psrep", bufs=1, space="PSUM") as psrep,
        ):
            # ---- constant loads -------------------------------------------
            wb_sb = consts.tile([128, FK, H], F32R)
            nc.sync.dma_start(out=wb_sb, in_=wb.rearrange("(k p) h -> p k h", p=128))
            wf_sb = consts.tile([128, FK, C2], F32R)
            nc.sync.dma_start(out=wf_sb, in_=wf.rearrange("(k p) c -> p k c", p=128))
            bfr_sb = consts.tile([1, C2], F32R)
            nc.sync.dma_start(out=bfr_sb, in_=bf_dup[:, :])
            id_sb = consts.tile([128, 128], F32R)
            nc.sync.dma_start(out=id_sb, in_=ident[:, :])
            ones_sb = consts.tile([1, PT], F32R)
            nc.sync.dma_start(out=ones_sb, in_=ones_row[:, :])

            # seq in both orientations, chunked so compute starts early
            seq_sb = consts.tile([128, FK, TLOC], F32R)
            seqt_sb = consts.tile([128, NT, F], F32R)
            seq_src = seq_s.rearrange("(k p) t -> p k t", p=128)
            seqt_src = seq_t.rearrange("(n p) f -> p n f", p=128)
            tchunk = TLOC // NSEQ_CHUNKS
            ntile_chunk = NT // NSEQ_CHUNKS
            for ci in range(NSEQ_CHUNKS):
                sl = slice(ci * tchunk, (ci + 1) * tchunk)
                nc.sync.dma_start(out=seq_sb[:, :, sl], in_=seq_src[:, :, sl])
                nsl = slice(ci * ntile_chunk, (ci + 1) * ntile_chunk)
                nc.sync.dma_start(out=seqt_sb[:, nsl, :], in_=seqt_src[:, nsl, :])

            # M^T[c,f] accumulator lives in PSUM across the whole t-loop
            ps_m = psm.tile([C, F], F32)

            # ---- main loop over 32 t-tiles --------------------------------
            for i in range(NT):
                ts = slice(i * PT, (i + 1) * PT)

                # logits+bias into PSUM: bias via K=1 ones-row matmul opens
                # the group, then 2 F-tiles of seq^T @ Wf (N=256 fast path)
                ps_lg = pslg.tile([PT, C2], F32)
                nc.tensor.matmul(ps_lg, ones_sb, bfr_sb, start=True, stop=False)
                for k in range(FK):
                    nc.tensor.matmul(
                        ps_lg, seq_sb[:, k, ts], wf_sb[:, k, :],
                        start=False, stop=(k == FK - 1),
                    )

                # softmax over classes (free dim) / T
                e = small.tile([PT, C], F32)
                nc.scalar.activation(
                    e, ps_lg[:, 0:C], mybir.ActivationFunctionType.Exp
                )
                ssum = small.tile([PT, 1], F32)
                nc.vector.reduce_sum(ssum, e, axis=mybir.AxisListType.X)
                rcp = small.tile([PT, 1], F32)
                nc.vector.reciprocal(rcp, ssum)
                att = small.tile([PT, C], F32R)
                nc.gpsimd.tensor_scalar(
                    att, e, rcp, 1.0 / T,
                    mybir.AluOpType.mult, mybir.AluOpType.mult,
                )
                nc.sync.dma_start(out=att_out[ts, :], in_=att)

                # M^T[c,f] += att[t,c].T @ seq^T[t,f]   (N=256 fast path)
                nc.tensor.matmul(
                    ps_m, att, seqt_sb[:, i, :],
                    start=(i == 0), stop=(i == NT - 1),
                )

            # ---- epilogue: rep0^T = M^T-transposed through Wb --------------
            mt_sb = consts.tile([C, F], F32R)
            nc.scalar.copy(mt_sb, ps_m)
            m_sb = consts.tile([128, FK, C], F32R)
            for k in range(FK):
                ps_t = pst.tile([128, C], F32R)
                nc.tensor.transpose(
                    ps_t, mt_sb[:, k * 128:(k + 1) * 128], id_sb
                )
                nc.vector.tensor_copy(m_sb[:, k, :], ps_t)

            ps_rep = psrep.tile([C, H], F32)
            for k in range(FK):
                for hb in range(H // HB):
                    hs = slice(hb * HB, (hb + 1) * HB)
                    nc.tensor.matmul(
                        ps_rep[:, hs], m_sb[:, k, :], wb_sb[:, k, hs],
                        start=(k == 0), stop=(k == FK - 1),
                    )
            rep_sb = consts.tile([C, H], F32)
            nc.scalar.copy(rep_sb, ps_rep)
            nc.sync.dma_start(out=rep_out[:, :], in_=rep_sb)

    nc.finalize()
    return nc


def _prepare_in_maps(seq, Wb, bb, Wa, ba):
    seq = np.ascontiguousarray(np.asarray(seq, dtype=np.float32))
    Wb = np.ascontiguousarray(np.asarray(Wb, dtype=np.float32))
    bb = np.asarray(bb, dtype=np.float32)
    Wa = np.asarray(Wa, dtype=np.float32)
    ba = np.asarray(ba, dtype=np.float32)

    wf1 = (Wb.astype(np.float64) @ Wa.astype(np.float64)).astype(np.float32)
    wf = np.ascontiguousarray(np.concatenate([wf1, wf1], axis=1))  # [F, 2C]
    bf = ((bb.astype(np.float64) @ Wa.astype(np.float64)
           + ba.astype(np.float64)).astype(np.float32))
    bf_dup = np.ascontiguousarray(np.concatenate([bf, bf])[None, :])  # [1, 2C]
    ident = np.eye(128, dtype=np.float32)
    ones_row = np.ones((1, PT), dtype=np.float32)

    in_maps = []
    for core in range(NCORES):
        b, sh = core // TSPLIT, core % TSPLIT
        t0 = sh * TLOC
        sl = seq[b, :, t0:t0 + TLOC]
        in_maps.append({
            "seq_s": np.ascontiguousarray(sl),
            "seq_t": np.ascontiguousarray(sl.T),
            "wb": Wb, "wf": wf, "bf_dup": bf_dup, "ident": ident,
            "ones_row": ones_row,
        })
    return in_maps


def _assemble(results, bb, action_matrix, action_bias, Wt, bt):
    bb = np.asarray(bb, dtype=np.float64)
    A = np.asarray(action_matrix, dtype=np.float64)
    action_bias = np.asarray(action_bias, dtype=np.float64)
    Wt = np.asarray(Wt, dtype=np.float64)
    bt = np.asarray(bt, dtype=np.float64)

    attention = np.empty((B, T, C), dtype=np.float32)
    rep_t = np.zeros((B, C, H), dtype=np.float64)
    for core in range(NCORES):
        r = results[core]
        b, sh = core // TSPLIT, core % TSPLIT
        t0 = sh * TLOC
        attention[b, t0:t0 + TLOC, :] = r["att_out"]
        rep_t[b] += r["rep_out"]

    # rank-1 bias correction: rep^T[c,h] += (sum_t att[t,c]) * bb[h]
    s_att = attention.astype(np.float64).sum(axis=1)          # [B, C]
    rep_t += s_att[:, :, None] * bb[None, None, :]

    rep_feature = np.ascontiguousarray(
        rep_t.transpose(0, 2, 1)).astype(np.float32)          # [B, H, C]
    rep64 = rep_feature.astype(np.float64)
    action_logit = (np.einsum("bhc,hc->bc", rep64, A)
                    + action_bias).astype(np.float32)
    thres = (np.einsum("bhc,h->bc", rep64, Wt[:, 0]) + bt).astype(np.float32)
    return attention, rep_feature, action_logit, thres


def run(inputs, **spmd_kwargs):
    """Build, run on 8 cores, and assemble. Returns (outputs, BassKernelResults)."""
    nc = build_nc()
    in_maps = _prepare_in_maps(
        inputs["seq"], inputs["Wb"], inputs["bb"], inputs["Wa"], inputs["ba"],
    )
    res = run_bass_kernel_spmd(nc, in_maps, core_ids=list(range(NCORES)),
                               **spmd_kwargs)
    outs = _assemble(res.results, inputs["bb"], inputs["action_matrix"],
                     inputs["action_bias"], inputs["Wt"], inputs["bt"])
    return outs, res


def kernel(**inputs):
    outs, _ = run(inputs)
    return outs


# revision 15
# speedup vs baseline: 1.2828x; 1.0725x over previous
"""Trainium2 Bass kernel for nn_AttentionNetwork (temporal attention pooling).

Reference computation (B=4, F=256, T=8192, H=1024, C=128):
    z         = einsum("bft,fh->bth", seq, Wb) + bb          [B,T,H]
    logits    = z @ Wa + ba                                   [B,T,C]
    attention = softmax(logits, axis=2) / T                   [B,T,C]
    rep       = einsum("bth,btc->bhc", z, attention)          [B,H,C]
    action    = einsum("bhc,hc->bc", rep, A) + action_bias    [B,C]
    thres     = (rep.transpose(0,2,1) @ Wt)[...,0] + bt       [B,C]

Sharding: 8 cores = 4 batch x 2 T-halves (T_loc = 4096 per core).

Key algebraic refactors (all exact up to fp reassociation):
  1. logits = seq^T @ (Wb@Wa) + (bb@Wa + ba)  -- Wf := Wb@Wa fused on host
     (F=256 contraction instead of H=1024, and z is not needed for logits).
  2. rep    = Wb^T @ (seq @ att) + outer(sum_t att, bb)
     -- contract seq with attention FIRST (matrix-chain reordering):
     M^T[c,f] = sum_t att[t,c] seq[f,t] accumulates tile-by-tile in PSUM,
     then one tiny projection through Wb at the end. z is never
     materialized at all; the host adds the rank-1 bb correction using
     sum_t att (computed from the attention output it already has).
  3. The logits bias rides a K=1 ones-row matmul into PSUM, so the
     softmax reads logits+bias straight from PSUM.

Per-core device work: logits (seq^T@Wf, N padded to 256 for the fp32r
fast path), softmax/T (ACT exp + DVE sum/recip + GPSIMD scale), M^T
accumulation, and the final Wb projection -- ~0.3 G MAC vs 1.74 G for
the naive z-based dataflow.

Matmuls run as float32r (fp32 stored, fp22 multiply, fp32 accumulate) --
4x the fp32 matmul rate on the PE array at moving-dim >= 256.

The host sends seq in BOTH orientations ([F,T_loc] for logits
stationaries, [T_loc,F] for the M^T matmul) -- a transposed copy is
cheaper as DMA than as on-device PE transposes.
"""

import numpy as np

import concourse.bacc as bacc
import concourse.mybir as mybir
import concourse.tile as tile
from concourse.bass_utils import run_bass_kernel_spmd

B, F, T, H, C = 4, 256, 8192, 1024, 128
NCORES = 8
TSPLIT = NCORES // B          # 2 T-shards per batch element
TLOC = T // TSPLIT            # 4096 timesteps per core
PT = 128                      # t-tile (partition dim)
NT = TLOC // PT               # 32 t-tiles
FK = F // 128                 # 2 contraction tiles over F
HB = 512                      # h-chunk per matmul (one PSUM bank, fp32)
NSEQ_CHUNKS = 8               # DMA pipelining chunks for the seq load

F32 = mybir.dt.float32
F32R = mybir.dt.float32r      # fp22 multiply / fp32 accumulate on PE
C2 = 2 * C                    # logits N padded to 256 (fp32r needs N>=256
                              # for the 1 cyc/row fast path; Wf cols duplicated)


def build_nc():
    nc = bacc.Bacc(trn_type="TRN2")

    # Per-core inputs (host pre-shards / pre-transposes / pre-duplicates).
    seq_s = nc.dram_tensor("seq_s", [F, TLOC], F32R, kind="ExternalInput")
    seq_t = nc.dram_tensor("seq_t", [TLOC, F], F32R, kind="ExternalInput")
    wb = nc.dram_tensor("wb", [F, H], F32R, kind="ExternalInput")
    wf = nc.dram_tensor("wf", [F, C2], F32R, kind="ExternalInput")
    expbf_bc = nc.dram_tensor("expbf_bc", [128, C], F32, kind="ExternalInput")
    ident = nc.dram_tensor("ident", [128, 128], F32R, kind="ExternalInput")

    att_out = nc.dram_tensor("att_out", [TLOC, C], F32R, kind="ExternalOutput")
    rep_out = nc.dram_tensor("rep_out", [C, H], F32, kind="ExternalOutput")

    with tile.TileContext(nc) as tc:
        with (
            tc.tile_pool(name="consts", bufs=1) as consts,
            tc.tile_pool(name="small", bufs=6) as small,
            tc.tile_pool(name="pslg", bufs=4, space="PSUM") as pslg,
            tc.tile_pool(name="psm", bufs=1, space="PSUM") as psm,
            tc.tile_pool(name="pst", bufs=1, space="PSUM") as pst,
            tc.tile_pool(name="psrep", bufs=1, space="PSUM") as psrep,
        ):
            # ---- constant loads -------------------------------------------
            wb_sb = consts.tile([128, FK, H], F32R)
            nc.sync.dma_start(out=wb_sb, in_=wb.rearrange("(k p) h -> p k h", p=128))
            wf_sb = consts.tile([128, FK, C2], F32R)
            nc.sync.dma_start(out=wf_sb, in_=wf.rearrange("(k p) c -> p k c", p=128))
            expbf_sb = consts.tile([128, C], F32)
            nc.sync.dma_start(out=expbf_sb, in_=expbf_bc[:, :])
            id_sb = consts.tile([128, 128], F32R)
            nc.sync.dma_start(out=id_sb, in_=ident[:, :])

            # seq in both orientations, chunked so compute starts early
            seq_sb = consts.tile([128, FK, TLOC], F32R)
            seqt_sb = consts.tile([128, NT, F], F32R)
            seq_src = seq_s.rearrange("(k p) t -> p k t", p=128)
            seqt_src = seq_t.rearrange("(n p) f -> p n f", p=128)
            tchunk = TLOC // NSEQ_CHUNKS
            ntile_chunk = NT // NSEQ_CHUNKS
            for ci in range(NSEQ_CHUNKS):
                sl = slice(ci * tchunk, (ci + 1) * tchunk)
                nc.sync.dma_start(out=seq_sb[:, :, sl], in_=seq_src[:, :, sl])
                nsl = slice(ci * ntile_chunk, (ci + 1) * ntile_chunk)
                nc.sync.dma_start(out=seqt_sb[:, nsl, :], in_=seqt_src[:, nsl, :])

            # M^T[c,f] accumulator lives in PSUM across the whole t-loop
            ps_m = psm.tile([C, F], F32)

            # ---- main loop over 32 t-tiles --------------------------------
            for i in range(NT):
                ts = slice(i * PT, (i + 1) * PT)

                # logits into PSUM: 2 F-tiles of seq^T @ Wf (N=256 fast path)
                ps_lg = pslg.tile([PT, C2], F32)
                for k in range(FK):
                    nc.tensor.matmul(
                        ps_lg, seq_sb[:, k, ts], wf_sb[:, k, :],
                        start=(k == 0), stop=(k == FK - 1),
                    )

                # softmax/T over classes (free dim), bias folded as
                # att = e*exp(bf) / sum_c(e*exp(bf)) / T with e = exp(logits)
                e = small.tile([PT, C], F32)
                nc.scalar.activation(
                    e, ps_lg[:, 0:C], mybir.ActivationFunctionType.Exp
                )
                em = small.tile([PT, C], F32)
                nc.gpsimd.tensor_mul(em, e, expbf_sb)
                ssum = small.tile([PT, 1], F32)
                nc.vector.reduce_sum(ssum, em, axis=mybir.AxisListType.X)
                rcp = small.tile([PT, 1], F32)
                nc.vector.reciprocal(rcp, ssum)
                att = small.tile([PT, C], F32R)
                nc.gpsimd.tensor_scalar(
                    att, em, rcp, 1.0 / T,
                    mybir.AluOpType.mult, mybir.AluOpType.mult,
                )
                nc.sync.dma_start(out=att_out[ts, :], in_=att)

                # M^T[c,f] += att[t,c].T @ seq^T[t,f]   (N=256 fast path)
                nc.tensor.matmul(
                    ps_m, att, seqt_sb[:, i, :],
                    start=(i == 0), stop=(i == NT - 1),
                )

            # ---- epilogue: rep0^T = M^T-transposed through Wb --------------
            mt_sb = consts.tile([C, F], F32R)
            nc.scalar.copy(mt_sb, ps_m)
            m_sb = consts.tile([128, FK, C], F32R)
            for k in range(FK):
                ps_t = pst.tile([128, C], F32R)
                nc.tensor.transpose(
                    ps_t, mt_sb[:, k * 128:(k + 1) * 128], id_sb
                )
                nc.vector.tensor_copy(m_sb[:, k, :], ps_t)

            ps_rep = psrep.tile([C, H], F32)
            for k in range(FK):
                for hb in range(H // HB):
                    hs = slice(hb * HB, (hb + 1) * HB)
                    nc.tensor.matmul(
                        ps_rep[:, hs], m_sb[:, k, :], wb_sb[:, k, hs],
                        start=(k == 0), stop=(k == FK - 1),
                    )
            rep_sb = consts.tile([C, H], F32)
            nc.scalar.copy(rep_sb, ps_rep)
            nc.sync.dma_start(out=rep_out[:, :], in_=rep_sb)

    nc.finalize()
    return nc


def _prepare_in_maps(seq, Wb, bb, Wa, ba):
    seq = np.ascontiguousarray(np.asarray(seq, dtype=np.float32))
    Wb = np.ascontiguousarray(np.asarray(Wb, dtype=np.float32))
    bb = np.asarray(bb, dtype=np.float32)
    Wa = np.asarray(Wa, dtype=np.float32)
    ba = np.asarray(ba, dtype=np.float32)

    wf1 = (Wb.astype(np.float64) @ Wa.astype(np.float64)).astype(np.float32)
    wf = np.ascontiguousarray(np.concatenate([wf1, wf1], axis=1))  # [F, 2C]
    bf = (bb.astype(np.float64) @ Wa.astype(np.float64)
          + ba.astype(np.float64))
    expbf = np.exp(bf).astype(np.float32)
    expbf_bc = np.ascontiguousarray(np.broadcast_to(expbf[None, :], (128, C)))
    ident = np.eye(128, dtype=np.float32)

    in_maps = []
    for core in range(NCORES):
        b, sh = core // TSPLIT, core % TSPLIT
        t0 = sh * TLOC
        sl = seq[b, :, t0:t0 + TLOC]
        in_maps.append({
            "seq_s": np.ascontiguousarray(sl),
            "seq_t": np.ascontiguousarray(sl.T),
            "wb": Wb, "wf": wf, "expbf_bc": expbf_bc, "ident": ident,
        })
    return in_maps


def _assemble(results, bb, action_matrix, action_bias, Wt, bt):
    bb = np.asarray(bb, dtype=np.float64)
    A = np.asarray(action_matrix, dtype=np.float64)
    action_bias = np.asarray(action_bias, dtype=np.float64)
    Wt = np.asarray(Wt, dtype=np.float64)
    bt = np.asarray(bt, dtype=np.float64)

    attention = np.empty((B, T, C), dtype=np.float32)
    rep_t = np.zeros((B, C, H), dtype=np.float64)
    for core in range(NCORES):
        r = results[core]
        b, sh = core // TSPLIT, core % TSPLIT
        t0 = sh * TLOC
        attention[b, t0:t0 + TLOC, :] = r["att_out"]
        rep_t[b] += r["rep_out"]

    # rank-1 bias correction: rep^T[c,h] += (sum_t att[t,c]) * bb[h]
    s_att = attention.astype(np.float64).sum(axis=1)          # [B, C]
    rep_t += s_att[:, :, None] * bb[None, None, :]

    rep_feature = np.ascontiguousarray(
        rep_t.transpose(0, 2, 1)).astype(np.float32)          # [B, H, C]
    rep64 = rep_feature.astype(np.float64)
    action_logit = (np.einsum("bhc,hc->bc", rep64, A)
                    + action_bias).astype(np.float32)
    thres = (np.einsum("bhc,h->bc", rep64, Wt[:, 0]) + bt).astype(np.float32)
    return attention, rep_feature, action_logit, thres


def run(inputs, **spmd_kwargs):
    """Build, run on 8 cores, and assemble. Returns (outputs, BassKernelResults)."""
    nc = build_nc()
    in_maps = _prepare_in_maps(
        inputs["seq"], inputs["Wb"], inputs["bb"], inputs["Wa"], inputs["ba"],
    )
    res = run_bass_kernel_spmd(nc, in_maps, core_ids=list(range(NCORES)),
                               **spmd_kwargs)
    outs = _assemble(res.results, inputs["bb"], inputs["action_matrix"],
                     inputs["action_bias"], inputs["Wt"], inputs["bt"])
    return outs, res


def kernel(**inputs):
    outs, _ = run(inputs)
    return outs


# revision 16
# speedup vs baseline: 1.3662x; 1.0650x over previous
"""Trainium2 Bass kernel for nn_AttentionNetwork (temporal attention pooling).

Reference computation (B=4, F=256, T=8192, H=1024, C=128):
    z         = einsum("bft,fh->bth", seq, Wb) + bb          [B,T,H]
    logits    = z @ Wa + ba                                   [B,T,C]
    attention = softmax(logits, axis=2) / T                   [B,T,C]
    rep       = einsum("bth,btc->bhc", z, attention)          [B,H,C]
    action    = einsum("bhc,hc->bc", rep, A) + action_bias    [B,C]
    thres     = (rep.transpose(0,2,1) @ Wt)[...,0] + bt       [B,C]

Sharding: 8 cores = 4 batch x 2 T-halves (T_loc = 4096 per core).

Key algebraic refactors (all exact up to fp reassociation):
  1. logits = seq^T @ (Wb@Wa) + (bb@Wa + ba)  -- Wf := Wb@Wa fused on host
     (F=256 contraction instead of H=1024, and z is not needed for logits).
  2. rep    = Wb^T @ (seq @ att) + outer(sum_t att, bb)
     -- contract seq with attention FIRST (matrix-chain reordering):
     M^T[c,f] = sum_t att[t,c] seq[f,t] accumulates tile-by-tile in PSUM,
     then one tiny projection through Wb at the end. z is never
     materialized at all; the host adds the rank-1 bb correction using
     sum_t att (computed from the attention output it already has).
  3. The logits bias rides a K=1 ones-row matmul into PSUM, so the
     softmax reads logits+bias straight from PSUM.

Per-core device work: logits (seq^T@Wf, N padded to 256 for the fp32r
fast path), softmax/T (ACT exp + DVE sum/recip + GPSIMD scale), M^T
accumulation, and the final Wb projection -- ~0.3 G MAC vs 1.74 G for
the naive z-based dataflow.

Matmuls run as float32r (fp32 stored, fp22 multiply, fp32 accumulate) --
4x the fp32 matmul rate on the PE array at moving-dim >= 256.

The host sends seq in BOTH orientations ([F,T_loc] for logits
stationaries, [T_loc,F] for the M^T matmul) -- a transposed copy is
cheaper as DMA than as on-device PE transposes.
"""

import numpy as np

import concourse.bacc as bacc
import concourse.mybir as mybir
import concourse.tile as tile
from concourse.bass_utils import run_bass_kernel_spmd

B, F, T, H, C = 4, 256, 8192, 1024, 128
NCORES = 8
TSPLIT = NCORES // B          # 2 T-shards per batch element
TLOC = T // TSPLIT            # 4096 timesteps per core
PT = 128                      # t-tile (partition dim)
NT = TLOC // PT               # 32 t-tiles
FK = F // 128                 # 2 contraction tiles over F
HB = 512                      # h-chunk per matmul (one PSUM bank, fp32)
NSEQ_CHUNKS = 16              # DMA pipelining chunks for the seq load

F32 = mybir.dt.float32
F32R = mybir.dt.float32r      # fp22 multiply / fp32 accumulate on PE
C2 = 2 * C                    # logits N padded to 256 (fp32r needs N>=256
                              # for the 1 cyc/row fast path; Wf cols duplicated)


def build_nc():
    nc = bacc.Bacc(trn_type="TRN2")

    # Per-core inputs (host pre-shards / pre-transposes / pre-duplicates).
    seq_s = nc.dram_tensor("seq_s", [F, TLOC], F32R, kind="ExternalInput")
    seq_t = nc.dram_tensor("seq_t", [TLOC, F], F32R, kind="ExternalInput")
    wb = nc.dram_tensor("wb", [F, H], F32R, kind="ExternalInput")
    wf = nc.dram_tensor("wf", [F, C2], F32R, kind="ExternalInput")
    expbf_bc = nc.dram_tensor("expbf_bc", [128, C], F32, kind="ExternalInput")
    ident = nc.dram_tensor("ident", [128, 128], F32R, kind="ExternalInput")

    att_out = nc.dram_tensor("att_out", [TLOC, C], F32R, kind="ExternalOutput")
    rep_out = nc.dram_tensor("rep_out", [C, H], F32, kind="ExternalOutput")

    with tile.TileContext(nc) as tc:
        with (
            tc.tile_pool(name="consts", bufs=1) as consts,
            tc.tile_pool(name="small", bufs=8) as small,
            tc.tile_pool(name="pslg", bufs=4, space="PSUM") as pslg,
            tc.tile_pool(name="psm", bufs=1, space="PSUM") as psm,
            tc.tile_pool(name="pst", bufs=1, space="PSUM") as pst,
            tc.tile_pool(name="psrep", bufs=1, space="PSUM") as psrep,
        ):
            # ---- constant loads -------------------------------------------
            # wf/expbf first (needed by tile 0); wb/ident only at the
            # epilogue -- load them on the scalar HWDGE queue so the sync
            # FIFO goes straight to seq chunks.
            wf_sb = consts.tile([128, FK, C2], F32R)
            nc.sync.dma_start(out=wf_sb, in_=wf.rearrange("(k p) c -> p k c", p=128))
            expbf_sb = consts.tile([128, C], F32)
            nc.sync.dma_start(out=expbf_sb, in_=expbf_bc[:, :])
            wb_sb = consts.tile([128, FK, H], F32R)
            nc.scalar.dma_start(out=wb_sb, in_=wb.rearrange("(k p) h -> p k h", p=128))
            id_sb = consts.tile([128, 128], F32R)
            nc.scalar.dma_start(out=id_sb, in_=ident[:, :])

            # seq in both orientations, chunked so compute starts early
            seq_sb = consts.tile([128, FK, TLOC], F32R)
            seqt_sb = consts.tile([128, NT, F], F32R)
            seq_src = seq_s.rearrange("(k p) t -> p k t", p=128)
            seqt_src = seq_t.rearrange("(n p) f -> p n f", p=128)
            tchunk = TLOC // NSEQ_CHUNKS
            ntile_chunk = NT // NSEQ_CHUNKS
            for ci in range(NSEQ_CHUNKS):
                sl = slice(ci * tchunk, (ci + 1) * tchunk)
                nc.sync.dma_start(out=seq_sb[:, :, sl], in_=seq_src[:, :, sl])
                nsl = slice(ci * ntile_chunk, (ci + 1) * ntile_chunk)
                nc.sync.dma_start(out=seqt_sb[:, nsl, :], in_=seqt_src[:, nsl, :])

            # M^T[c,f] accumulator lives in PSUM across the whole t-loop
            ps_m = psm.tile([C, F], F32)

            # ---- main loop over 32 t-tiles --------------------------------
            for i in range(NT):
                ts = slice(i * PT, (i + 1) * PT)

                # logits into PSUM: 2 F-tiles of seq^T @ Wf (N=256 fast path)
                ps_lg = pslg.tile([PT, C2], F32)
                for k in range(FK):
                    nc.tensor.matmul(
                        ps_lg, seq_sb[:, k, ts], wf_sb[:, k, :],
                        start=(k == 0), stop=(k == FK - 1),
                    )

                # softmax/T over classes (free dim), bias folded as
                # att = e*exp(bf) / sum_c(e*exp(bf)) / T with e = exp(logits)
                e = small.tile([PT, C], F32)
                nc.scalar.activation(
                    e, ps_lg[:, 0:C], mybir.ActivationFunctionType.Exp
                )
                em = small.tile([PT, C], F32)
                nc.vector.tensor_mul(em, e, expbf_sb)
                ssum = small.tile([PT, 1], F32)
                nc.vector.reduce_sum(ssum, em, axis=mybir.AxisListType.X)
                rcp = small.tile([PT, 1], F32)
                nc.vector.reciprocal(rcp, ssum)
                att = small.tile([PT, C], F32R)
                nc.gpsimd.tensor_scalar(
                    att, em, rcp, 1.0 / T,
                    mybir.AluOpType.mult, mybir.AluOpType.mult,
                )
                nc.sync.dma_start(out=att_out[ts, :], in_=att)

                # M^T[c,f] += att[t,c].T @ seq^T[t,f]   (N=256 fast path)
                nc.tensor.matmul(
                    ps_m, att, seqt_sb[:, i, :],
                    start=(i == 0), stop=(i == NT - 1),
                )

            # ---- epilogue: rep0^T = M^T-transposed through Wb --------------
            mt_sb = consts.tile([C, F], F32R)
            nc.scalar.copy(mt_sb, ps_m)
            m_sb = consts.tile([128, FK, C], F32R)
            for k in range(FK):
                ps_t = pst.tile([128, C], F32R)
                nc.tensor.transpose(
                    ps_t, mt_sb[:, k * 128:(k + 1) * 128], id_sb
                )
                nc.vector.tensor_copy(m_sb[:, k, :], ps_t)

            ps_rep = psrep.tile([C, H], F32)
            for k in range(FK):
                for hb in range(H // HB):
                    hs = slice(hb * HB, (hb + 1) * HB)
                    nc.tensor.matmul(
                        ps_rep[:, hs], m_sb[:, k, :], wb_sb[:, k, hs],
                        start=(k == 0), stop=(k == FK - 1),
                    )
            rep_sb = consts.tile([C, H], F32)
            nc.scalar.copy(rep_sb, ps_rep)
            nc.sync.dma_start(out=rep_out[:, :], in_=rep_sb)

    nc.finalize()
    return nc


def _prepare_in_maps(seq, Wb, bb, Wa, ba):
    seq = np.ascontiguousarray(np.asarray(seq, dtype=np.float32))
    Wb = np.ascontiguousarray(np.asarray(Wb, dtype=np.float32))
    bb = np.asarray(bb, dtype=np.float32)
    Wa = np.asarray(Wa, dtype=np.float32)
    ba = np.asarray(ba, dtype=np.float32)

    wf1 = (Wb.astype(np.float64) @ Wa.astype(np.float64)).astype(np.float32)
    wf = np.ascontiguousarray(np.concatenate([wf1, wf1], axis=1))  # [F, 2C]
    bf = (bb.astype(np.float64) @ Wa.astype(np.float64)
          + ba.astype(np.float64))
    expbf = np.exp(bf).astype(np.float32)
    expbf_bc = np.ascontiguousarray(np.broadcast_to(expbf[None, :], (128, C)))
    ident = np.eye(128, dtype=np.float32)

    in_maps = []
    for core in range(NCORES):
        b, sh = core // TSPLIT, core % TSPLIT
        t0 = sh * TLOC
        sl = seq[b, :, t0:t0 + TLOC]
        in_maps.append({
            "seq_s": np.ascontiguousarray(sl),
            "seq_t": np.ascontiguousarray(sl.T),
            "wb": Wb, "wf": wf, "expbf_bc": expbf_bc, "ident": ident,
        })
    return in_maps


def _assemble(results, bb, action_matrix, action_bias, Wt, bt):
    bb = np.asarray(bb, dtype=np.float64)
    A = np.asarray(action_matrix, dtype=np.float64)
    action_bias = np.asarray(action_bias, dtype=np.float64)
    Wt = np.asarray(Wt, dtype=np.float64)
    bt = np.asarray(bt, dtype=np.float64)

    attention = np.empty((B, T, C), dtype=np.float32)
    rep_t = np.zeros((B, C, H), dtype=np.float64)
    for core in range(NCORES):
        r = results[core]
        b, sh = core // TSPLIT, core % TSPLIT
        t0 = sh * TLOC
        attention[b, t0:t0 + TLOC, :] = r["att_out"]
        rep_t[b] += r["rep_out"]

    # rank-1 bias correction: rep^T[c,h] += (sum_t att[t,c]) * bb[h]
    s_att = attention.astype(np.float64).sum(axis=1)          # [B, C]
    rep_t += s_att[:, :, None] * bb[None, None, :]

    rep_feature = np.ascontiguousarray(
        rep_t.transpose(0, 2, 1)).astype(np.float32)          # [B, H, C]
    rep64 = rep_feature.astype(np.float64)
    action_logit = (np.einsum("bhc,hc->bc", rep64, A)
                    + action_bias).astype(np.float32)
    thres = (np.einsum("bhc,h->bc", rep64, Wt[:, 0]) + bt).astype(np.float32)
    return attention, rep_feature, action_logit, thres


def run(inputs, **spmd_kwargs):
    """Build, run on 8 cores, and assemble. Returns (outputs, BassKernelResults)."""
    nc = build_nc()
    in_maps = _prepare_in_maps(
        inputs["seq"], inputs["Wb"], inputs["bb"], inputs["Wa"], inputs["ba"],
    )
    res = run_bass_kernel_spmd(nc, in_maps, core_ids=list(range(NCORES)),
                               **spmd_kwargs)
    outs = _assemble(res.results, inputs["bb"], inputs["action_matrix"],
                     inputs["action_bias"], inputs["Wt"], inputs["bt"])
    return outs, res


def kernel(**inputs):
    outs, _ = run(inputs)
    return outs


# revision 17
# speedup vs baseline: 1.3951x; 1.0211x over previous
"""Trainium2 Bass kernel for nn_AttentionNetwork (temporal attention pooling).

Reference computation (B=4, F=256, T=8192, H=1024, C=128):
    z         = einsum("bft,fh->bth", seq, Wb) + bb          [B,T,H]
    logits    = z @ Wa + ba                                   [B,T,C]
    attention = softmax(logits, axis=2) / T                   [B,T,C]
    rep       = einsum("bth,btc->bhc", z, attention)          [B,H,C]
    action    = einsum("bhc,hc->bc", rep, A) + action_bias    [B,C]
    thres     = (rep.transpose(0,2,1) @ Wt)[...,0] + bt       [B,C]

Sharding: 8 cores = 4 batch x 2 T-halves (T_loc = 4096 per core).

Key algebraic refactors (all exact up to fp reassociation):
  1. logits = seq^T @ (Wb@Wa) + (bb@Wa + ba)  -- Wf := Wb@Wa fused on host
     (F=256 contraction instead of H=1024, and z is not needed for logits).
  2. rep    = Wb^T @ (seq @ att) + outer(sum_t att, bb)
     -- contract seq with attention FIRST (matrix-chain reordering):
     M^T[c,f] = sum_t att[t,c] seq[f,t] accumulates tile-by-tile in PSUM,
     then one tiny projection through Wb at the end. z is never
     materialized at all; the host adds the rank-1 bb correction using
     sum_t att (computed from the attention output it already has).
  3. The logits bias rides a K=1 ones-row matmul into PSUM, so the
     softmax reads logits+bias straight from PSUM.

Per-core device work: logits (seq^T@Wf, N padded to 256 for the fp32r
fast path), softmax/T (ACT exp + DVE sum/recip + GPSIMD scale), M^T
accumulation, and the final Wb projection -- ~0.3 G MAC vs 1.74 G for
the naive z-based dataflow.

Matmuls run as float32r (fp32 stored, fp22 multiply, fp32 accumulate) --
4x the fp32 matmul rate on the PE array at moving-dim >= 256.

The host sends seq in BOTH orientations ([F,T_loc] for logits
stationaries, [T_loc,F] for the M^T matmul) -- a transposed copy is
cheaper as DMA than as on-device PE transposes.
"""

import numpy as np

import concourse.bacc as bacc
import concourse.mybir as mybir
import concourse.tile as tile
from concourse.bass_utils import run_bass_kernel_spmd

B, F, T, H, C = 4, 256, 8192, 1024, 128
NCORES = 8
TSPLIT = NCORES // B          # 2 T-shards per batch element
TLOC = T // TSPLIT            # 4096 timesteps per core
PT = 128                      # t-tile (partition dim)
NT = TLOC // PT               # 32 t-tiles
FK = F // 128                 # 2 contraction tiles over F
HB = 512                      # h-chunk per matmul (one PSUM bank, fp32)
NSEQ_CHUNKS = 16              # DMA pipelining chunks for the seq load

F32 = mybir.dt.float32
F32R = mybir.dt.float32r      # fp22 multiply / fp32 accumulate on PE
C2 = 2 * C                    # logits N padded to 256 (fp32r needs N>=256
                              # for the 1 cyc/row fast path; Wf cols duplicated)


def build_nc():
    nc = bacc.Bacc(trn_type="TRN2")

    # Per-core inputs (host pre-shards / pre-transposes / pre-duplicates).
    seq_s = nc.dram_tensor("seq_s", [F, TLOC], F32R, kind="ExternalInput")
    seq_t = nc.dram_tensor("seq_t", [TLOC, F], F32R, kind="ExternalInput")
    wb = nc.dram_tensor("wb", [F, H], F32R, kind="ExternalInput")
    wf = nc.dram_tensor("wf", [F, C2], F32R, kind="ExternalInput")
    expbf_bc = nc.dram_tensor("expbf_bc", [128, C2], F32, kind="ExternalInput")
    ident = nc.dram_tensor("ident", [128, 128], F32R, kind="ExternalInput")

    att_out = nc.dram_tensor("att_out", [TLOC, C], F32R, kind="ExternalOutput")
    rep_out = nc.dram_tensor("rep_out", [C, H], F32, kind="ExternalOutput")

    with tile.TileContext(nc) as tc:
        with (
            tc.tile_pool(name="consts", bufs=1) as consts,
            tc.tile_pool(name="small", bufs=8) as small,
            tc.tile_pool(name="pslg", bufs=4, space="PSUM") as pslg,
            tc.tile_pool(name="psm", bufs=1, space="PSUM") as psm,
            tc.tile_pool(name="pst", bufs=1, space="PSUM") as pst,
            tc.tile_pool(name="psrep", bufs=1, space="PSUM") as psrep,
        ):
            # ---- constant loads -------------------------------------------
            # wf/expbf first (needed by tile 0); wb/ident only at the
            # epilogue -- load them on the scalar HWDGE queue so the sync
            # FIFO goes straight to seq chunks.
            wf_sb = consts.tile([128, FK, C2], F32R)
            nc.sync.dma_start(out=wf_sb, in_=wf.rearrange("(k p) c -> p k c", p=128))
            expbf_sb = consts.tile([128, C2], F32)
            nc.sync.dma_start(out=expbf_sb, in_=expbf_bc[:, :])

            # seq in both orientations, chunked so compute starts early
            seq_sb = consts.tile([128, FK, TLOC], F32R)
            seqt_sb = consts.tile([128, NT, F], F32R)
            seq_src = seq_s.rearrange("(k p) t -> p k t", p=128)
            seqt_src = seq_t.rearrange("(n p) f -> p n f", p=128)
            tchunk = TLOC // NSEQ_CHUNKS
            ntile_chunk = NT // NSEQ_CHUNKS
            for ci in range(NSEQ_CHUNKS):
                sl = slice(ci * tchunk, (ci + 1) * tchunk)
                nc.sync.dma_start(out=seq_sb[:, :, sl], in_=seq_src[:, :, sl])
                nsl = slice(ci * ntile_chunk, (ci + 1) * ntile_chunk)
                nc.sync.dma_start(out=seqt_sb[:, nsl, :], in_=seqt_src[:, nsl, :])

            # epilogue-only constants load after the seq stream
            wb_sb = consts.tile([128, FK, H], F32R)
            nc.sync.dma_start(out=wb_sb, in_=wb.rearrange("(k p) h -> p k h", p=128))
            id_sb = consts.tile([128, 128], F32R)
            nc.sync.dma_start(out=id_sb, in_=ident[:, :])

            # M^T[c,f] accumulator lives in PSUM across the whole t-loop
            ps_m = psm.tile([C, F], F32)

            # ---- main loop over 32 t-tiles, processed in pairs ------------
            for ip in range(NT // 2):
                e2 = small.tile([PT, 2, C], F32)
                lgs = []
                for j in range(2):
                    i = 2 * ip + j
                    ts = slice(i * PT, (i + 1) * PT)
                    # logits into PSUM: 2 F-tiles of seq^T @ Wf (N=256)
                    ps_lg = pslg.tile([PT, C2], F32)
                    for k in range(FK):
                        nc.tensor.matmul(
                            ps_lg, seq_sb[:, k, ts], wf_sb[:, k, :],
                            start=(k == 0), stop=(k == FK - 1),
                        )
                    nc.scalar.activation(
                        e2[:, j, :], ps_lg[:, 0:C],
                        mybir.ActivationFunctionType.Exp
                    )
                    lgs.append(ps_lg)

                # batched softmax pieces for the pair (DVE)
                em2 = small.tile([PT, 2, C], F32)
                nc.vector.tensor_mul(em2, e2, expbf_sb.rearrange("p (j c) -> p j c", j=2))
                ssum2 = small.tile([PT, 2], F32)
                nc.vector.reduce_sum(ssum2, em2, axis=mybir.AxisListType.X)
                rcp2 = small.tile([PT, 2], F32)
                nc.vector.reciprocal(rcp2, ssum2)

                for j in range(2):
                    i = 2 * ip + j
                    ts = slice(i * PT, (i + 1) * PT)
                    att = small.tile([PT, C], F32R)
                    nc.gpsimd.tensor_scalar(
                        att, em2[:, j, :], rcp2[:, j:j + 1], 1.0 / T,
                        mybir.AluOpType.mult, mybir.AluOpType.mult,
                    )
                    nc.sync.dma_start(out=att_out[ts, :], in_=att)
                    # M^T[c,f] += att[t,c].T @ seq^T[t,f]   (N=256)
                    nc.tensor.matmul(
                        ps_m, att, seqt_sb[:, i, :],
                        start=(i == 0), stop=(i == NT - 1),
                    )

            # ---- epilogue: rep0^T = M^T-transposed through Wb --------------
            mt_sb = consts.tile([C, F], F32R)
            nc.scalar.copy(mt_sb, ps_m)
            m_sb = consts.tile([128, FK, C], F32R)
            for k in range(FK):
                ps_t = pst.tile([128, C], F32R)
                nc.tensor.transpose(
                    ps_t, mt_sb[:, k * 128:(k + 1) * 128], id_sb
                )
                nc.vector.tensor_copy(m_sb[:, k, :], ps_t)

            ps_rep = psrep.tile([C, H], F32)
            for k in range(FK):
                for hb in range(H // HB):
                    hs = slice(hb * HB, (hb + 1) * HB)
                    nc.tensor.matmul(
                        ps_rep[:, hs], m_sb[:, k, :], wb_sb[:, k, hs],
                        start=(k == 0), stop=(k == FK - 1),
                    )
            rep_sb = consts.tile([C, H], F32)
            nc.scalar.copy(rep_sb[:, 0:HB], ps_rep[:, 0:HB])
            nc.vector.tensor_copy(rep_sb[:, HB:H], ps_rep[:, HB:H])
            nc.sync.dma_start(out=rep_out[:, :], in_=rep_sb)

    nc.finalize()
    return nc


def _prepare_in_maps(seq, Wb, bb, Wa, ba):
    seq = np.ascontiguousarray(np.asarray(seq, dtype=np.float32))
    Wb = np.ascontiguousarray(np.asarray(Wb, dtype=np.float32))
    bb = np.asarray(bb, dtype=np.float32)
    Wa = np.asarray(Wa, dtype=np.float32)
    ba = np.asarray(ba, dtype=np.float32)

    wf1 = (Wb.astype(np.float64) @ Wa.astype(np.float64)).astype(np.float32)
    wf = np.ascontiguousarray(np.concatenate([wf1, wf1], axis=1))  # [F, 2C]
    bf = (bb.astype(np.float64) @ Wa.astype(np.float64)
          + ba.astype(np.float64))
    expbf = np.exp(bf).astype(np.float32)
    expbf2 = np.concatenate([expbf, expbf])
    expbf_bc = np.ascontiguousarray(np.broadcast_to(expbf2[None, :], (128, C2)))
    ident = np.eye(128, dtype=np.float32)

    in_maps = []
    for core in range(NCORES):
        b, sh = core // TSPLIT, core % TSPLIT
        t0 = sh * TLOC
        sl = seq[b, :, t0:t0 + TLOC]
        in_maps.append({
            "seq_s": np.ascontiguousarray(sl),
            "seq_t": np.ascontiguousarray(sl.T),
            "wb": Wb, "wf": wf, "expbf_bc": expbf_bc, "ident": ident,
        })
    return in_maps


def _assemble(results, bb, action_matrix, action_bias, Wt, bt):
    bb = np.asarray(bb, dtype=np.float64)
    A = np.asarray(action_matrix, dtype=np.float64)
    action_bias = np.asarray(action_bias, dtype=np.float64)
    Wt = np.asarray(Wt, dtype=np.float64)
    bt = np.asarray(bt, dtype=np.float64)

    attention = np.empty((B, T, C), dtype=np.float32)
    rep_t = np.zeros((B, C, H), dtype=np.float64)
    for core in range(NCORES):
        r = results[core]
        b, sh = core // TSPLIT, core % TSPLIT
        t0 = sh * TLOC
        attention[b, t0:t0 + TLOC, :] = r["att_out"]
        rep_t[b] += r["rep_out"]

    # rank-1 bias correction: rep^T[c,h] += (sum_t att[t,c]) * bb[h]
    s_att = attention.astype(np.float64).sum(axis=1)          # [B, C]
    rep_t += s_att[:, :, None] * bb[None, None, :]

    rep_feature = np.ascontiguousarray(
        rep_t.transpose(0, 2, 1)).astype(np.float32)          # [B, H, C]
    rep64 = rep_feature.astype(np.float64)
    action_logit = (np.einsum("bhc,hc->bc", rep64, A)
                    + action_bias).astype(np.float32)
    thres = (np.einsum("bhc,h->bc", rep64, Wt[:, 0]) + bt).astype(np.float32)
    return attention, rep_feature, action_logit, thres


def run(inputs, **spmd_kwargs):
    """Build, run on 8 cores, and assemble. Returns (outputs, BassKernelResults)."""
    nc = build_nc()
    in_maps = _prepare_in_maps(
        inputs["seq"], inputs["Wb"], inputs["bb"], inputs["Wa"], inputs["ba"],
    )
    res = run_bass_kernel_spmd(nc, in_maps, core_ids=list(range(NCORES)),
                               **spmd_kwargs)
    outs = _assemble(res.results, inputs["bb"], inputs["action_matrix"],
                     inputs["action_bias"], inputs["Wt"], inputs["bt"])
    return outs, res


def kernel(**inputs):
    outs, _ = run(inputs)
    return outs
